# revision 28
# baseline (speedup 1.0000x reference)
"""NeuralFingerprint GNN message passing on 8 TRN2 NeuronCores (v2).

Sharding: each degree bucket split contiguously 8 ways (65536 rows/core,
bucket-major local order); weights replicated; BN stats AllReduced; conv0's
BN+ReLU'd output x AllGathered (fp8) so conv1 can gather arbitrary rows.

Key structure vs the v1 baseline (5.38 ms -> ~1.8 ms):
- conv0 neighbor gathers are host-permuted into a transposed linear stream
  nfeT [80, 179200] bf16 (16 edge + 64 node rows, consumption order), so
  stage A has zero indirect DMAs and zero on-device transposes.
- conv0's per-group edge sums (edge rows of the summed stream) are saved to
  DRAM (esT) and reused by conv1 -- the edge stream is read and summed once.
- All matmuls run in bf16 (4x PE throughput vs the f32 baseline).
- x is stored fp8(e4m3) for the AllGather + conv1 neighbor gathers (halves
  the exposed collective); the self path / heads keep bf16 x.
- conv1 gathers are [128, 1]-offset indirect DMAs (one row per partition per
  instruction -- multi-offset APs silently misbehave on real HW), ~1400 ops
  that dominate the middle of the schedule on the GpSimd queue.
- Softmax heads 0/1 (which depend only on nf / x) are computed during the
  conv1 gather window and stored normalized (sm01T); the post-BN1 tail only
  computes head 2 and combines with one fused scalar_tensor_tensor per chunk.
- DMA traffic is batched into 2048-col slabs, spread across the SP/ACT
  queues, with strided 3-level APs for the row-major x_rm / acc writes.
"""
import sys
import numpy as np

sys.path.insert(0, "/opt/trn_rl_repo")

N = 524288
E = 1433600
NODE, EDGE, H, OUT = 64, 16, 128, 128
BN_EPS = 1e-5
NCORES = 8
CNT = [8192, 65536, 131072, 196608, 98304, 24576]          # rows per degree d=0..5
PC = [c // NCORES for c in CNT]                            # per-core rows per degree
ROWS = sum(PC)                                             # 65536
GPD = [p // 512 for p in PC]                               # groups of 512 per degree
NG = sum(GPD)                                              # 128
START = [0, 8192, 73728, 204800, 401408, 499712]           # global bucket starts
LSTART = [0, 1024, 9216, 25600, 50176, 62464]              # local bucket starts
IDXW = 4 * sum(GPD[d] * d for d in range(6))               # 1400 packed idx cols
PCE = sum(PC[d] * d for d in range(6))                     # 179200 gathered rows/core
NCH = 1                                                    # AllGather chunks
CR = ROWS // NCH                                           # local rows per AG chunk
SLAB = 4                                                   # groups per DMA slab
NSLAB = NG // SLAB


def _group_meta():
    """[(d, idx_off, col_off)] per group: idx_off = nidx1 column base,
    col_off = nfeT column base."""
    meta, ioff, coff = [], 0, 0
    for d in range(6):
        for _ in range(GPD[d]):
            meta.append((d, ioff, coff))
            if d > 0:
                ioff += 4 * d
                coff += 512 * d
    return meta


def _build_nc(debug=False):
    from concourse import bass, bacc, mybir
    import concourse.tile as tile
    from concourse.masks import make_identity

    f32, i32, bf16 = mybir.dt.float32, mybir.dt.int32, mybir.dt.bfloat16
    f8 = mybir.dt.float8e4
    AF = mybir.ActivationFunctionType
    ALU = mybir.AluOpType
    AX = mybir.AxisListType
    nc = bacc.Bacc("TRN2", target_bir_lowering=False)

    # nfeT/wc0 rows are edge-first: [0:16] edge features, [16:80] node features
    nfeT = nc.dram_tensor("nfeT", [EDGE + NODE, PCE], bf16, kind="ExternalInput")
    nfT = nc.dram_tensor("nfT", [NODE, ROWS], bf16, kind="ExternalInput")
    nidx1 = nc.dram_tensor("nidx1", [128, IDXW], i32, kind="ExternalInput")
    ws0 = nc.dram_tensor("ws0", [NODE, H], bf16, kind="ExternalInput")
    wc0 = nc.dram_tensor("wc0", [EDGE + NODE, 5 * H], bf16, kind="ExternalInput")
    ws1 = nc.dram_tensor("ws1", [H, H], bf16, kind="ExternalInput")
    wn1 = nc.dram_tensor("wn1", [H, 5 * H], bf16, kind="ExternalInput")
    we1 = nc.dram_tensor("we1", [EDGE, 5 * H], bf16, kind="ExternalInput")
    w0b = nc.dram_tensor("w0b", [NODE, OUT], bf16, kind="ExternalInput")
    w1b = nc.dram_tensor("w1b", [H, OUT], bf16, kind="ExternalInput")
    w2b = nc.dram_tensor("w2b", [H, OUT], bf16, kind="ExternalInput")
    acc = nc.dram_tensor("acc", [ROWS, OUT], f32, kind="ExternalOutput")
    if debug:
        dbg = {
            "a0T": nc.dram_tensor("dbg_a0T", [128, ROWS], bf16, kind="ExternalOutput"),
            "esT": nc.dram_tensor("dbg_esT", [EDGE, ROWS], bf16, kind="ExternalOutput"),
            "xTd": nc.dram_tensor("dbg_xTd", [128, ROWS], bf16, kind="ExternalOutput"),
            "x_rm": nc.dram_tensor("dbg_x_rm", [ROWS, H], f8, kind="ExternalOutput"),
            "x_full": nc.dram_tensor("dbg_x_full", [N, H], f8, kind="ExternalOutput"),
            "a1T": nc.dram_tensor("dbg_a1T", [128, ROWS], bf16, kind="ExternalOutput"),
        }

    meta = _group_meta()
    RG = [list(range(NCORES))]

    with tile.TileContext(nc) as tc:
        with (
            tc.tile_pool(name="pers", bufs=1) as pers,
            tc.tile_pool(name="drp", bufs=1, space="DRAM") as drp,
            tc.tile_pool(name="iop", bufs=8) as iop,
            tc.tile_pool(name="gp", bufs=8) as gp,
            tc.tile_pool(name="sp2", bufs=8) as sp2,
            tc.tile_pool(name="pp", bufs=2, space="PSUM") as pp,
            tc.tile_pool(name="pt", bufs=4, space="PSUM") as pt,
        ):
            # ---- DRAM staging ----
            a0T = drp.tile([128, ROWS], bf16, name="a0T")
            esT = drp.tile([EDGE, ROWS], bf16, name="esT")
            xTd = drp.tile([128, ROWS], bf16, name="xTd")
            x_rm = drp.tile([ROWS, H], f8, name="x_rm")
            x_full = drp.tile([N, H], f8, addr_space="Shared", name="x_full")
            sm01T = drp.tile([128, ROWS], bf16, name="sm01T")
            a1T = drp.tile([128, ROWS], bf16, name="a1T")
            cc0i = drp.tile([128, 2], f32, name="cc0i")
            cc0o = drp.tile([128, 2], f32, addr_space="Shared", name="cc0o")
            cc1i = drp.tile([128, 2], f32, name="cc1i")
            cc1o = drp.tile([128, 2], f32, addr_space="Shared", name="cc1o")

            # ---- persistent SBUF ----
            def pload(dram, shape, dtype, name):
                t = pers.tile(shape, dtype, name=name)
                nc.sync.dma_start(out=t[:], in_=dram[:])
                return t

            ws0_s = pload(ws0, [NODE, H], bf16, "ws0_s")
            wc0_s = pload(wc0, [EDGE + NODE, 5 * H], bf16, "wc0_s")
            ws1_s = pload(ws1, [H, H], bf16, "ws1_s")
            wn1_s = pload(wn1, [H, 5 * H], bf16, "wn1_s")
            we1_s = pload(we1, [EDGE, 5 * H], bf16, "we1_s")
            w0_s = pload(w0b, [NODE, OUT], bf16, "w0_s")
            w1_s = pload(w1b, [H, OUT], bf16, "w1_s")
            w2_s = pload(w2b, [H, OUT], bf16, "w2_s")
            ni1 = pload(nidx1, [128, IDXW], i32, "ni1")
            identb = pers.tile([128, 128], bf16, name="identb")
            make_identity(nc, identb[:])
            sum0 = pers.tile([128, NG], f32, name="sum0")
            ssq0 = pers.tile([128, NG], f32, name="ssq0")
            sum1 = pers.tile([128, NG], f32, name="sum1")
            ssq1 = pers.tile([128, NG], f32, name="ssq1")
            junk = pers.tile([128, NG], f32, name="junk")

            def bn_params(sum_t, ssq_t, cci, cco, tag):
                tot = pers.tile([128, 2], f32, name=f"tot{tag}")
                nc.scalar.activation(out=junk[:], in_=sum_t[:], func=AF.Copy,
                                     accum_out=tot[:, 0:1])
                nc.scalar.activation(out=junk[:], in_=ssq_t[:], func=AF.Copy,
                                     accum_out=tot[:, 1:2])
                nc.scalar.dma_start(out=cci[:], in_=tot[:])
                nc.gpsimd.collective_compute(
                    "AllReduce", ALU.add, replica_groups=RG,
                    ins=[cci[:]], outs=[cco[:]])
                rt = pers.tile([128, 2], f32, name=f"rt{tag}")
                nc.scalar.dma_start(out=rt[:], in_=cco[:])
                mean = pers.tile([128, 1], f32, name=f"mean{tag}")
                nc.vector.tensor_scalar_mul(out=mean[:], in0=rt[:, 0:1], scalar1=1.0 / N)
                var = pers.tile([128, 1], f32, name=f"var{tag}")
                nc.vector.tensor_scalar_mul(out=var[:], in0=rt[:, 1:2], scalar1=1.0 / N)
                m2 = pers.tile([128, 1], f32, name=f"m2{tag}")
                nc.scalar.square(out=m2[:], in_=mean[:])
                nc.vector.tensor_scalar_mul(out=m2[:], in0=m2[:], scalar1=-1.0)
                nc.vector.tensor_add(out=var[:], in0=var[:], in1=m2[:])
                nc.vector.tensor_scalar_add(out=var[:], in0=var[:], scalar1=BN_EPS)
                std = pers.tile([128, 1], f32, name=f"std{tag}")
                nc.scalar.sqrt(out=std[:], in_=var[:])
                istd = pers.tile([128, 1], f32, name=f"istd{tag}")
                nc.vector.reciprocal(out=istd[:], in_=std[:])
                bnb = pers.tile([128, 1], f32, name=f"bnb{tag}")
                nc.vector.tensor_scalar_mul(out=bnb[:], in0=mean[:], scalar1=-1.0)
                nc.vector.tensor_scalar_mul(out=bnb[:], in0=bnb[:], scalar1=istd[:, 0:1])
                return istd, bnb

            # ================= Stage A: conv0 =================
            for sl in range(NSLAB):
                g0 = sl * SLAB
                nf4 = iop.tile([NODE, SLAB * 512], bf16, name="nf4", tag="nf4",
                               bufs=2)
                nc.sync.dma_start(out=nf4[:], in_=nfT[:, g0 * 512:(g0 + SLAB) * 512])
                ab4 = iop.tile([128, SLAB * 512], bf16, name="ab4", tag="ab4",
                               bufs=3)
                es4 = sp2.tile([EDGE, SLAB * 512], bf16, name="es4", tag="es4",
                               bufs=2)
                for gi in range(SLAB):
                    gg = g0 + gi
                    d, _, coff = meta[gg]
                    csl = slice(gi * 512, (gi + 1) * 512)
                    act = pp.tile([128, 512], f32, name="act", tag="mm")
                    if d == 0:
                        nc.tensor.matmul(act[:], ws0_s[:], nf4[:, csl],
                                         start=True, stop=True)
                        nc.vector.memset(es4[:, csl], 0.0)
                    else:
                        gt = gp.tile([EDGE + NODE, 2560], bf16, name="gt",
                                     tag="gt", bufs=3)
                        nc.sync.dma_start(out=gt[:, 0:512 * d],
                                          in_=nfeT[:, coff:coff + 512 * d])
                        for j in range(1, d):
                            nc.vector.tensor_add(
                                out=gt[:, 0:512], in0=gt[:, 0:512],
                                in1=gt[:, j * 512:(j + 1) * 512])
                        nc.tensor.matmul(act[:], ws0_s[:], nf4[:, csl],
                                         start=True, stop=False)
                        nc.tensor.matmul(act[:], wc0_s[:, (d - 1) * H:d * H],
                                         gt[:, 0:512], start=False, stop=True)
                        # edge rows are 0:16 (edge-first layout) -> same
                        # partitions as es4, so a DVE copy is legal
                        nc.vector.tensor_copy(es4[:, csl], gt[0:EDGE, 0:512])
                    nc.scalar.activation(out=ab4[:, csl], in_=act[:], func=AF.Copy,
                                         accum_out=sum0[:, gg:gg + 1])
                    sq = iop.tile([128, 512], bf16, name="sq", tag="sq", bufs=3)
                    nc.vector.scalar_tensor_tensor(
                        out=sq[:], in0=ab4[:, csl], scalar=0.0, in1=ab4[:, csl],
                        op0=ALU.bypass, op1=ALU.mult,
                        accum_out=ssq0[:, gg:gg + 1])
                nc.scalar.dma_start(out=a0T[:, g0 * 512:(g0 + SLAB) * 512],
                                    in_=ab4[:])
                nc.scalar.dma_start(out=esT[:, g0 * 512:(g0 + SLAB) * 512],
                                    in_=es4[:])

            istd0, bnb0 = bn_params(sum0, ssq0, cc0i, cc0o, "0")

            # ================= Stage A3: BN+ReLU, xTd + x_rm, chunked AG ====
            from concourse.bass import AP as BassAP
            gpc = NG // NCH                      # groups per AG chunk
            for ch in range(NCH):
                for sl in range(gpc // SLAB):
                    g0 = ch * gpc + sl * SLAB
                    a_in = iop.tile([128, SLAB * 512], bf16, name="a_in",
                                    tag="ab4", bufs=3)
                    nc.sync.dma_start(out=a_in[:],
                                      in_=a0T[:, g0 * 512:(g0 + SLAB) * 512])
                    xtb = iop.tile([128, SLAB * 512], bf16, name="xtb",
                                   tag="xtb", bufs=3)
                    nc.scalar.activation(out=xtb[:], in_=a_in[:], func=AF.Relu,
                                         bias=bnb0[:, 0:1], scale=istd0[:, 0:1])
                    nc.scalar.dma_start(out=xTd[:, g0 * 512:(g0 + SLAB) * 512],
                                        in_=xtb[:])
                    xs4 = sp2.tile([128, SLAB * 512], f8, name="xs4",
                                   tag="xs4", bufs=2)
                    for q in range(SLAB * 4):
                        px = pt.tile([128, 128], bf16, name="px", tag="tp")
                        nc.tensor.transpose(px[:], xtb[:, q * 128:(q + 1) * 128],
                                            identb[:])
                        nc.vector.tensor_copy(xs4[:, q * 128:(q + 1) * 128], px[:])
                    # strided write: row g0*512 + q*128 + p, col f <- xs4[p, q*128+f]
                    out_ap = BassAP(
                        x_rm[:].tensor, g0 * 512 * H,
                        [(H, 128), (128 * H, SLAB * 4), (1, H)])
                    nc.sync.dma_start(out=out_ap, in_=xs4[:])
                nc.gpsimd.collective_compute(
                    "AllGather", ALU.bypass, replica_groups=RG,
                    ins=[x_rm[ch * CR:(ch + 1) * CR, :]],
                    outs=[x_full[ch * 8 * CR:(ch + 1) * 8 * CR, :]])

            # ===== Stage B: conv1 + head-0/1 softmax precompute =============
            # heads 0/1 depend only on nf/x, so they run here, hidden behind
            # the gpsimd gather stream; B3 only handles head 2 + combine.
            for sl in range(NSLAB):
                g0 = sl * SLAB
                xo4 = iop.tile([H, SLAB * 512], bf16, name="xo4", tag="nf4",
                               bufs=2)
                nc.sync.dma_start(out=xo4[:], in_=xTd[:, g0 * 512:(g0 + SLAB) * 512])
                nf4 = iop.tile([NODE, SLAB * 512], bf16, name="nf4b", tag="nfb",
                               bufs=2)
                nc.sync.dma_start(out=nf4[:], in_=nfT[:, g0 * 512:(g0 + SLAB) * 512])
                eb4 = sp2.tile([EDGE, SLAB * 512], bf16, name="eb4", tag="es4",
                               bufs=2)
                nc.scalar.dma_start(out=eb4[:], in_=esT[:, g0 * 512:(g0 + SLAB) * 512])
                ab4 = iop.tile([128, SLAB * 512], bf16, name="ab14", tag="ab4",
                               bufs=3)
                sm4 = sp2.tile([128, SLAB * 512], bf16, name="sm4", tag="sm4",
                               bufs=3)
                for gi in range(SLAB):
                    gg = g0 + gi
                    d, ioff, _ = meta[gg]
                    csl = slice(gi * 512, (gi + 1) * 512)
                    act1 = pp.tile([128, 512], f32, name="act1", tag="mm")
                    if d == 0:
                        nc.tensor.matmul(act1[:], ws1_s[:], xo4[:, csl],
                                         start=True, stop=True)
                    else:
                        gt1 = gp.tile([128, 2560], f8, name="gt1", tag="g1",
                                      bufs=6)
                        for m in range(4 * d):
                            nc.gpsimd.indirect_dma_start(
                                out=gt1[:, m * H:(m + 1) * H], out_offset=None,
                                in_=x_full[:],
                                in_offset=bass.IndirectOffsetOnAxis(
                                    ap=ni1[:, ioff + m:ioff + m + 1], axis=0))
                        gtc = gp.tile([128, 2560], bf16, name="gtc", tag="g1c",
                                      bufs=4)
                        nc.scalar.copy(out=gtc[:, 0:4 * d * H],
                                       in_=gt1[:, 0:4 * d * H])
                        rn = sp2.tile([H, 512], bf16, name="rn", tag="rn", bufs=4)
                        for c in range(4):
                            b0 = c * d * H
                            for j in range(1, d):
                                nc.vector.tensor_add(
                                    out=gtc[:, b0:b0 + H], in0=gtc[:, b0:b0 + H],
                                    in1=gtc[:, b0 + j * H:b0 + (j + 1) * H])
                            pt1 = pt.tile([128, 128], bf16, name="pt1", tag="tp")
                            nc.tensor.transpose(pt1[:], gtc[:, b0:b0 + H], identb[:])
                            nc.scalar.copy(out=rn[:, c * 128:(c + 1) * 128],
                                           in_=pt1[:])
                        nc.tensor.matmul(act1[:], ws1_s[:], xo4[:, csl],
                                         start=True, stop=False)
                        nc.tensor.matmul(act1[:], wn1_s[:, (d - 1) * H:d * H],
                                         rn[:], start=False, stop=False)
                        nc.tensor.matmul(act1[:], we1_s[:, (d - 1) * H:d * H],
                                         eb4[:, csl], start=False, stop=True)
                    nc.scalar.activation(out=ab4[:, csl], in_=act1[:], func=AF.Copy,
                                         accum_out=sum1[:, gg:gg + 1])
                    sq1 = iop.tile([128, 512], bf16, name="sq1", tag="sq", bufs=3)
                    nc.vector.scalar_tensor_tensor(
                        out=sq1[:], in0=ab4[:, csl], scalar=0.0, in1=ab4[:, csl],
                        op0=ALU.bypass, op1=ALU.mult,
                        accum_out=ssq1[:, gg:gg + 1])
                    # heads 0/1 for this group (softmax0 + softmax1, normalized)
                    for c in range(4):
                        qs = slice((gi * 4 + c) * 128, (gi * 4 + c + 1) * 128)
                        ps2 = pp.tile([128, 2, 128], f32, name="ps2", tag="h01")
                        nc.tensor.matmul(ps2[:, 0, :], nf4[:, qs], w0_s[:],
                                         start=True, stop=True)
                        nc.tensor.matmul(ps2[:, 1, :], xo4[:, qs], w1_s[:],
                                         start=True, stop=True)
                        esb2 = sp2.tile([128, 2, 128], bf16, name="esb2",
                                        tag="esb", bufs=3)
                        nc.scalar.activation(out=esb2[:], in_=ps2[:], func=AF.Exp)
                        den2 = sp2.tile([128, 2], f32, name="den2", tag="den",
                                        bufs=3)
                        nc.vector.tensor_reduce(den2[:], esb2[:], AX.X, ALU.add)
                        rec2 = sp2.tile([128, 2], f32, name="rec2", tag="rec",
                                        bufs=3)
                        nc.vector.reciprocal(out=rec2[:], in_=den2[:])
                        ssl = sm4[:, qs]
                        nc.vector.tensor_scalar_mul(
                            out=ssl, in0=esb2[:, 0, :], scalar1=rec2[:, 0:1])
                        nc.vector.scalar_tensor_tensor(
                            out=ssl, in0=esb2[:, 1, :], scalar=rec2[:, 1:2],
                            in1=ssl, op0=ALU.mult, op1=ALU.add)
                nc.scalar.dma_start(out=a1T[:, g0 * 512:(g0 + SLAB) * 512],
                                    in_=ab4[:])
                nc.scalar.dma_start(out=sm01T[:, g0 * 512:(g0 + SLAB) * 512],
                                    in_=sm4[:])

            istd1, bnb1 = bn_params(sum1, ssq1, cc1i, cc1o, "1")

            if debug:
                nc.sync.dma_start(out=dbg["a0T"][:], in_=a0T[:])
                nc.sync.dma_start(out=dbg["esT"][:], in_=esT[:])
                nc.sync.dma_start(out=dbg["xTd"][:], in_=xTd[:])
                nc.sync.dma_start(out=dbg["x_rm"][:], in_=x_rm[:])
                nc.sync.dma_start(out=dbg["x_full"][:], in_=x_full[:])
                nc.sync.dma_start(out=dbg["a1T"][:], in_=a1T[:])

            # ========== Stage B3: BN+ReLU + head 2 + combine with sm01 ======
            for sl in range(NSLAB):
                g0 = sl * SLAB
                sm4b = iop.tile([128, SLAB * 512], bf16, name="sm4b", tag="xtb",
                                bufs=3)
                nc.sync.dma_start(out=sm4b[:],
                                  in_=sm01T[:, g0 * 512:(g0 + SLAB) * 512])
                ai4 = iop.tile([128, SLAB * 512], bf16, name="ai4", tag="ab4",
                               bufs=3)
                nc.sync.dma_start(out=ai4[:], in_=a1T[:, g0 * 512:(g0 + SLAB) * 512])
                yt4 = iop.tile([128, SLAB * 512], bf16, name="yt4", tag="yt4",
                               bufs=3)
                nc.scalar.activation(out=yt4[:], in_=ai4[:], func=AF.Relu,
                                     bias=bnb1[:, 0:1], scale=istd1[:, 0:1])
                for gi in range(SLAB):
                    gg = g0 + gi
                    acc_g = sp2.tile([128, 512], f32, name="acc_g", tag="accs",
                                     bufs=3)
                    for c in range(4):
                        qs = slice((gi * 4 + c) * 128, (gi * 4 + c + 1) * 128)
                        z2 = pp.tile([128, 512], f32, name="z2", tag="mm")
                        nc.tensor.matmul(z2[:, 0:128], yt4[:, qs], w2_s[:],
                                         start=True, stop=True)
                        esb3 = sp2.tile([128, 128], bf16, name="esb3", tag="esb3",
                                        bufs=3)
                        den3 = sp2.tile([128, 1], f32, name="den3", tag="den",
                                        bufs=3)
                        nc.scalar.activation(out=esb3[:], in_=z2[:, 0:128], func=AF.Exp)
                        nc.vector.reduce_sum(out=den3[:], in_=esb3[:], axis=AX.X)
                        rec3 = sp2.tile([128, 1], f32, name="rec3", tag="rec",
                                        bufs=3)
                        nc.vector.reciprocal(out=rec3[:], in_=den3[:])
                        # acc = softmax0 + softmax1 + exp2 * rec2 in one DVE op
                        nc.vector.scalar_tensor_tensor(
                            out=acc_g[:, c * 128:(c + 1) * 128], in0=esb3[:],
                            scalar=rec3[:, 0:1], in1=sm4b[:, qs],
                            op0=ALU.mult, op1=ALU.add)
                    # strided write: row gg*512 + c*128 + p, col f <- acc_g[p, c*128+f]
                    acc_ap = BassAP(
                        acc[:].tensor, gg * 512 * OUT,
                        [(OUT, 128), (128 * OUT, 4), (1, OUT)])
                    nc.sync.dma_start(out=acc_ap, in_=acc_g[:])
    return nc


class _Runner:
    def __init__(self, nc, n_cores):
        import jax
        from jax.experimental.shard_map import shard_map
        from jax.sharding import Mesh, PartitionSpec, NamedSharding
        from concourse import bass2jax, mybir
        from concourse.bass2jax import _bass_exec_p, install_neuronx_cc_hook

        install_neuronx_cc_hook()
        if hasattr(nc, "is_finalized") and not nc.is_finalized():
            nc.finalize()
        self.jax = jax
        self.n_cores = n_cores
        partition_name = nc.partition_id_tensor.name if nc.partition_id_tensor else None
        in_names, out_names, out_avals, zero_outs = [], [], [], []
        for alloc in nc.m.functions[0].allocations:
            if not isinstance(alloc, mybir.MemoryLocationSet):
                continue
            name = alloc.memorylocations[0].name
            if alloc.kind == "ExternalInput":
                if name != partition_name:
                    in_names.append(name)
            elif alloc.kind == "ExternalOutput":
                out_names.append(name)
                shape = tuple(alloc.tensor_shape)
                dtype = mybir.dt.np(alloc.dtype)
                out_avals.append(jax.core.ShapedArray(shape, dtype))
                zero_outs.append(np.zeros(shape, dtype))
        self.in_names, self.out_names, self.zero_outs = in_names, out_names, zero_outs
        all_in_names = in_names + out_names
        if partition_name:
            all_in_names = all_in_names + [partition_name]

        def _body(*args):
            operands = list(args)
            if partition_name:
                operands.append(bass2jax.partition_id_tensor())
            return tuple(
                _bass_exec_p.bind(
                    *operands,
                    out_avals=tuple(out_avals),
                    in_names=tuple(all_in_names),
                    out_names=tuple(out_names),
                    lowering_input_output_aliases=(),
                    sim_require_finite=True,
                    sim_require_nnan=True,
                    nc=nc,
                )
            )

        nio = len(in_names) + len(out_names)
        devices = jax.devices()[:n_cores]
        self.mesh = Mesh(np.asarray(devices), ("core",))
        self.sharding = NamedSharding(self.mesh, PartitionSpec("core"))
        self.fn = jax.jit(
            shard_map(
                _body,
                mesh=self.mesh,
                in_specs=(PartitionSpec("core"),) * nio,
                out_specs=(PartitionSpec("core"),) * len(out_names),
                check_rep=False,
            ),
            keep_unused=True,
        )
        self._dev_args = None

    def stage(self, in_maps):
        args = [
            np.concatenate(
                [np.asarray(in_maps[c][n]) for c in range(self.n_cores)], axis=0
            )
            for n in self.in_names
        ] + [np.concatenate([z] * self.n_cores, axis=0) for z in self.zero_outs]
        self._dev_args = [self.jax.device_put(a, self.sharding) for a in args]
        return self

    def call(self):
        out = self.fn(*self._dev_args)
        self.jax.block_until_ready(out)
        return out

    def results(self, out):
        res = []
        for c in range(self.n_cores):
            m = {}
            for i, name in enumerate(self.out_names):
                full = np.asarray(out[i])
                per = full.shape[0] // self.n_cores
                m[name] = full[c * per:(c + 1) * per]
            res.append(m)
        return res


def _own_rows(k):
    rows = np.empty(ROWS, np.int64)
    for b in range(6):
        rows[LSTART[b]:LSTART[b] + PC[b]] = START[b] + k * PC[b] + np.arange(PC[b])
    return rows


def _remap_to_ag(o):
    """Global node id -> row in x_full (chunked AllGather layout)."""
    starts = np.array(START, np.int64)
    b = np.searchsorted(starts, o, side="right") - 1
    rel = o - starts[b]
    pcs = np.array(PC, np.int64)[b]
    core = rel // pcs
    loc = np.array(LSTART, np.int64)[b] + rel % pcs
    ch = loc // CR
    within = loc % CR
    return (ch * (NCORES * CR) + core * CR + within).astype(np.int32)


def _pack_idx(arrs_by_d, k):
    """[128, IDXW] index pack for core k: column ioff(g) + c*d + j holds, at
    partition p, the j-th neighbor of local node g*512 + c*128 + p."""
    out = np.empty((128, IDXW), np.int32)
    off = 0
    for d in range(1, 6):
        a = arrs_by_d[d][k * PC[d]:(k + 1) * PC[d]]
        for g in range(GPD[d]):
            sub = a[g * 512:(g + 1) * 512]
            blk = sub.reshape(4, 128, d).transpose(1, 0, 2).reshape(128, 4 * d)
            out[:, off:off + 4 * d] = blk
            off += 4 * d
    return out


def _build_nfeT(nf, ef, nn, ne, k, bf16):
    """[80, PCE] combined transposed neighbor stream for core k.

    Edge-first rows: [0:16] edge feats, [16:80] node feats. For group g the
    columns [coff + j*512 + r] hold neighbor j of group-local node r."""
    out = np.empty((EDGE + NODE, PCE), bf16)
    coff = 0
    for d in range(1, 6):
        a = nn[d][k * PC[d]:(k + 1) * PC[d]]          # [PCd, d]
        e = ne[d][k * PC[d]:(k + 1) * PC[d]]
        gn = nf[a]                                     # [PCd, d, 64]
        ge = ef[e]                                     # [PCd, d, 16]
        blk = np.concatenate([ge, gn], axis=2)         # [PCd, d, 80] edge-first
        # per group g: [512, d, 80] -> [80, d, 512] -> [80, d*512]
        blk = blk.reshape(GPD[d], 512, d, EDGE + NODE).transpose(0, 3, 2, 1)
        blk = blk.reshape(GPD[d], EDGE + NODE, d * 512).astype(bf16)
        w = d * 512
        for g in range(GPD[d]):
            out[:, coff:coff + w] = blk[g]
            coff += w
    return out


_STATE = {}


def _get_runner():
    if "runner" not in _STATE:
        _STATE["runner"] = _Runner(_build_nc(), NCORES)
    return _STATE["runner"]


def _prepare_in_maps(inputs):
    import ml_dtypes
    bf16 = ml_dtypes.bfloat16
    nf = np.ascontiguousarray(np.asarray(inputs["node_feat"], np.float32))
    ef = np.ascontiguousarray(np.asarray(inputs["edge_feat"], np.float32))
    nn = {d: np.asarray(inputs[f"nbr_node_d{d}"], np.int32) for d in range(1, 6)}
    ne = {d: np.asarray(inputs[f"nbr_edge_d{d}"], np.int32) for d in range(1, 6)}
    nn_ag = {d: _remap_to_ag(nn[d].astype(np.int64)) for d in range(1, 6)}

    wd0 = np.asarray(inputs["conv0_degW"], np.float32)
    wd1 = np.asarray(inputs["conv1_degW"], np.float32)
    shared = {
        "ws0": np.ascontiguousarray(
            np.asarray(inputs["conv0_selfW"], np.float32).T).astype(bf16),
        "wc0": np.ascontiguousarray(
            np.concatenate(
                [np.concatenate([wd0[d][:, NODE:], wd0[d][:, :NODE]], axis=1).T
                 for d in range(5)], axis=1)).astype(bf16),
        "ws1": np.ascontiguousarray(
            np.asarray(inputs["conv1_selfW"], np.float32).T).astype(bf16),
        "wn1": np.ascontiguousarray(
            np.concatenate([wd1[d][:, :H].T for d in range(5)], axis=1)
        ).astype(bf16),
        "we1": np.ascontiguousarray(
            np.concatenate([wd1[d][:, H:].T for d in range(5)], axis=1)
        ).astype(bf16),
        "w0b": np.ascontiguousarray(
            np.asarray(inputs["out_W0"], np.float32).T).astype(bf16),
        "w1b": np.ascontiguousarray(
            np.asarray(inputs["out_W1"], np.float32).T).astype(bf16),
        "w2b": np.ascontiguousarray(
            np.asarray(inputs["out_W2"], np.float32).T).astype(bf16),
    }
    in_maps = []
    for k in range(NCORES):
        m = dict(shared)
        m["nfT"] = np.ascontiguousarray(nf[_own_rows(k)].T).astype(bf16)
        m["nfeT"] = _build_nfeT(nf, ef, nn, ne, k, bf16)
        m["nidx1"] = _pack_idx(nn_ag, k)
        in_maps.append(m)
    return in_maps


def kernel(**inputs):
    in_maps = _prepare_in_maps(inputs)
    runner = _get_runner().stage(in_maps)
    out = runner.call()
    res = runner.results(out)
    acc_global = np.empty((N, OUT), np.float32)
    for k in range(NCORES):
        acc_global[_own_rows(k)] = res[k]["acc"]
    return acc_global[np.asarray(inputs["atom_index"])]
